# revision 9
# baseline (speedup 1.0000x reference)
"""Trainium2 Bass kernel for NeuroplasticLlama block-sparse adapter (moe_routing).

Contract: kernel(**inputs) takes FULL unsharded inputs (as produced by
setup_inputs) and returns the FULL [4, 4096, 4096] float32 output.

Strategy (data/sequence parallel over 8 cores, 2048 tokens each):
  - Each core's 2048 contiguous tokens belong to exactly one batch, so the
    task embedding contributes only per-core constant bias vectors
    (te @ A, te @ W2) -- h = x + te is never materialized.
  - The whole routed computation is made dense:
      scores s[t,n] = x @ (Wp @ centers.T)[:,n] + const_n   (per-token shift
        dropped; softmax over top-k and the top-k set are shift invariant)
      top-3 selection via threshold = 3rd max (3 rounds of max + mask-out)
      gates g[t,n] = exp(s - max) * (s >= thr3) / sum(...)
      z[t,:] (all 512 block-rank pairs) = x @ A_all  (dense)
      zg = z * expand4(g);  delta = block-diag(Bm) matmul
  - The device computes only DELTA (fp8 in, fp8 out); the residual
    y = x + delta is applied on the host during unsharding, so the x term
    keeps full f32 precision and HBM traffic per core is 2 MB x-in +
    2 MB delta-out per 512-token macrotile.
  - x is fed pre-transposed [H, tokens] fp8e4m3; scores and z both run as
    fp8 DoubleRow matmuls off the same tile (fp32 scores cost 127us/core
    of PE time in the previous version; fp8 scores ~14us with rel err
    1.4e-3 vs the 2e-2 budget -- routing flips only happen on near-ties
    where the softmax gates make the difference negligible).
  - delta matmul is bf16 (DoubleRow would not help: it is output-stream
    bound at 512 cols/chunk), PSUM -> SBUF fp8 copies alternate between
    the Scalar and Vector engines to keep both under the PE's ~26us/mt.
"""

import sys

if "/opt/trn_rl_repo" not in sys.path:
    sys.path.insert(0, "/opt/trn_rl_repo")

import numpy as np
import ml_dtypes

H = 4096
NB = 128
BLK = 32
R = 4
B = 4
S = 4096
NCORES = 8
TPC = (B * S) // NCORES  # tokens per core = 2048
T = 512                  # tokens per macrotile
NMT = TPC // T           # 4 macrotiles per core
NKT = H // 128           # 32 k-tiles over the hidden dim
NP = NKT // 2            # 16 DoubleRow k-pair tiles
BIG = 1.0e30

TRACE = False            # set by test.py for profiling runs
TRACE_DIR = None
LAST_RESULT = None       # BassKernelResults of the last run

_COMPILED = None


def _build():
    import concourse.bacc as bacc
    import concourse.tile as tile
    from concourse import mybir, masks

    f32 = mybir.dt.float32
    bf16 = mybir.dt.bfloat16
    f8 = mybir.dt.float8e4
    AF = mybir.ActivationFunctionType
    AL = mybir.AluOpType
    AX = mybir.AxisListType
    DR = mybir.MatmulPerfMode.DoubleRow

    nc = bacc.Bacc("TRN2", target_bir_lowering=False, debug=False,
                   num_devices=NCORES)

    xt_d = nc.dram_tensor("xt", [H, TPC], f8, kind="ExternalInput")
    ah_d = nc.dram_tensor("ah", [128, 4 * NKT * 128], f8, kind="ExternalInput")
    ws_d = nc.dram_tensor("ws", [128, NKT * 128], f8, kind="ExternalInput")
    bpk_d = nc.dram_tensor("bpk", [128, NKT * 128], bf16, kind="ExternalInput")
    e_d = nc.dram_tensor("e", [128, 512], bf16, kind="ExternalInput")
    bias_d = nc.dram_tensor("bias", [128, 5], f32, kind="ExternalInput")
    yt_d = nc.dram_tensor("yt", [H, TPC], f8, kind="ExternalOutput")

    xt_ap = xt_d.ap()
    yt_ap = yt_d.ap()

    with tile.TileContext(nc) as tc:
        from contextlib import ExitStack
        with ExitStack() as ctx:
            cpool = ctx.enter_context(tc.tile_pool(name="consts", bufs=1))
            xpool = ctx.enter_context(tc.tile_pool(name="xg", bufs=2))
            dpool = ctx.enter_context(tc.tile_pool(name="dall", bufs=2))
            zpool = ctx.enter_context(tc.tile_pool(name="zb", bufs=8))
            gpool = ctx.enter_context(tc.tile_pool(name="gate", bufs=3))
            spool = ctx.enter_context(tc.tile_pool(name="scal", bufs=4))
            pp = ctx.enter_context(tc.tile_pool(name="ps", bufs=2, space="PSUM"))

            # ---- persistent constants ----
            ws = cpool.tile([128, NKT * 128], f8, name="ws", tag="ws")
            nc.gpsimd.dma_start(ws[:], ws_d.ap()[:])
            az = []
            for q in range(4):
                t_az = cpool.tile([128, NKT * 128], f8, name=f"az{q}",
                                  tag=f"az{q}")
                nc.gpsimd.dma_start(t_az[:], ah_d.ap()[:, q * NKT * 128:(q + 1) * NKT * 128])
                az.append(t_az)
            bpk = cpool.tile([128, NKT * 128], bf16, name="bpk", tag="bpk")
            nc.gpsimd.dma_start(bpk[:], bpk_d.ap()[:])
            esb = cpool.tile([128, 512], bf16, name="esb", tag="esb")
            nc.gpsimd.dma_start(esb[:], e_d.ap()[:])
            bias = cpool.tile([128, 5], f32, name="bias", tag="bias")
            nc.gpsimd.dma_start(bias[:], bias_d.ap()[:])
            ident = cpool.tile([128, 128], bf16, name="ident", tag="ident")
            masks.make_identity(nc, ident[:])

            NTS = T // 128  # token sub-tiles per macrotile

            for mt in range(NMT):
                t0 = mt * T
                # ---- load x macrotile (fp8, one 2MB DMA) ----
                xa = xpool.tile([128, NKT * T], f8, name="xa", tag="xa")
                nc.sync.dma_start(
                    xa.rearrange("p (k t) -> p k t", k=NKT),
                    xt_ap[:, t0:t0 + T].rearrange("(k p) t -> p k t", p=128),
                )
                # delta output staging tile
                da = dpool.tile([128, NKT * T], f8, name="da", tag="da")

                # ---- scores chunk (fp8 DoubleRow): sT[n, t] ----
                sp = pp.tile([128, T], f32, space="PSUM", name="sp", tag="zp")
                for k2 in range(NP):
                    nc.tensor.matmul(
                        sp[:],
                        ws[:, k2 * 256:(k2 + 1) * 256]
                        .rearrange("p (two m) -> p two m", two=2),
                        xa[:, 2 * k2 * T:(2 * k2 + 2) * T]
                        .rearrange("p (two t) -> p two t", two=2),
                        start=(k2 == 0), stop=(k2 == NP - 1),
                        perf_mode=DR,
                    )
                s_sb = gpool.tile([128, T], bf16, name="s_sb", tag="s_sb")
                nc.scalar.activation(s_sb[:], sp[:], AF.Identity,
                                     bias=bias[:, 4:5], scale=1.0)

                # ---- gating phase A: transpose score subtiles ----
                stns = []
                for ts in range(NTS):
                    s_ps = pp.tile([128, 128], bf16, space="PSUM", name="s_ps",
                                   tag="tr", bufs=1)
                    nc.tensor.transpose(s_ps[:], s_sb[:, ts * 128:(ts + 1) * 128],
                                        ident[:])
                    stn = gpool.tile([128, 128], f32, name="stn", tag="stn",
                                     bufs=NTS + 1)
                    nc.scalar.copy(stn[:], s_ps[:])
                    stns.append(stn)

                # ---- gating phase B: DVE chain (overlaps z matmuls) ----
                ggs = []
                for ts in range(NTS):
                    stn = stns[ts]
                    r1 = spool.tile([128, 1], f32, name="r1", tag="r1")
                    nc.vector.reduce_max(r1[:], stn[:], axis=AX.X)
                    mb1 = gpool.tile([128, 128], f32, name="mb1", tag="mb1")
                    nc.vector.tensor_scalar(mb1[:], stn[:], r1[:], BIG,
                                            AL.is_ge, AL.mult)
                    s2 = gpool.tile([128, 128], f32, name="s2", tag="s2")
                    nc.vector.tensor_sub(s2[:], stn[:], mb1[:])
                    r2 = spool.tile([128, 1], f32, name="r2", tag="r2")
                    nc.vector.reduce_max(r2[:], s2[:], axis=AX.X)
                    mb2 = gpool.tile([128, 128], f32, name="mb2", tag="mb2")
                    nc.vector.tensor_scalar(mb2[:], s2[:], r2[:], BIG,
                                            AL.is_ge, AL.mult)
                    s3 = gpool.tile([128, 128], f32, name="s3", tag="s3")
                    nc.vector.tensor_sub(s3[:], s2[:], mb2[:])
                    r3 = spool.tile([128, 1], f32, name="r3", tag="r3")
                    nc.vector.reduce_max(r3[:], s3[:], axis=AX.X)
                    nr1 = spool.tile([128, 1], f32, name="nr1", tag="nr1")
                    nc.vector.tensor_scalar_mul(nr1[:], r1[:], -1.0)
                    ex = gpool.tile([128, 128], f32, name="ex", tag="ex")
                    nc.scalar.activation(ex[:], stn[:], AF.Exp, bias=nr1[:],
                                         scale=1.0)
                    em = gpool.tile([128, 128], f32, name="em", tag="em")
                    zs = spool.tile([128, 1], f32, name="zs", tag="zs")
                    nc.vector.scalar_tensor_tensor(em[:], stn[:], r3[:], ex[:],
                                                   AL.is_ge, AL.mult,
                                                   accum_out=zs[:])
                    rz = spool.tile([128, 1], f32, name="rz", tag="rz")
                    nc.vector.reciprocal(rz[:], zs[:])
                    gg = gpool.tile([128, 128], bf16, name="gg", tag="gg",
                                    bufs=NTS + 1)
                    nc.vector.tensor_scalar_mul(gg[:], em[:], rz[:])
                    ggs.append(gg)

                # ---- z chunks (fp8 DoubleRow) ----
                zbs = []
                for q in range(4):
                    zp = pp.tile([128, T], f32, space="PSUM", name="zp", tag="zp")
                    for k2 in range(NP):
                        nc.tensor.matmul(
                            zp[:],
                            az[q][:, k2 * 256:(k2 + 1) * 256]
                            .rearrange("p (two m) -> p two m", two=2),
                            xa[:, 2 * k2 * T:(2 * k2 + 2) * T]
                            .rearrange("p (two t) -> p two t", two=2),
                            start=(k2 == 0), stop=(k2 == NP - 1),
                            perf_mode=DR,
                        )
                    zb = zpool.tile([128, T], bf16, name="zb", tag="zb")
                    nc.scalar.activation(zb[:], zp[:], AF.Identity,
                                         bias=bias[:, q:q + 1], scale=1.0)
                    zbs.append(zb)

                # ---- gating phase C: transpose gates back ----
                gt_sb = gpool.tile([128, T], bf16, name="gt_sb", tag="gt_sb")
                for ts in range(NTS):
                    g_ps = pp.tile([128, 128], bf16, space="PSUM", name="g_ps",
                                   tag="tr", bufs=1)
                    nc.tensor.transpose(g_ps[:], ggs[ts][:], ident[:])
                    nc.scalar.copy(gt_sb[:, ts * 128:(ts + 1) * 128], g_ps[:])

                # ---- per quarter: expand gates, apply, delta, store ----
                # 32 PSUM->SBUF fp8 copies split ~22 scalar / 10 vector so
                # both engines stay under the PE's per-mt budget
                VEC_HL = {3, 6}
                for q in range(4):
                    gx = pp.tile([128, T], f32, space="PSUM", name="gx", tag="gx",
                                 bufs=1)
                    nc.tensor.matmul(gx[:],
                                     esb[:, q * 128:(q + 1) * 128],
                                     gt_sb[:],
                                     start=True, stop=True)
                    nc.vector.tensor_mul(zbs[q][:], zbs[q][:], gx[:])
                    for hl in range(8):
                        hc = q * 8 + hl
                        dp = pp.tile([128, T], f32, space="PSUM", name="dp",
                                     tag="dp", bufs=4)
                        nc.tensor.matmul(dp[:],
                                         bpk[:, hc * 128:(hc + 1) * 128],
                                         zbs[q][:],
                                         start=True, stop=True)
                        dsl = da[:, hc * T:(hc + 1) * T]
                        if hl in VEC_HL or (q >= 2 and hl == 1):
                            nc.vector.tensor_copy(dsl, dp[:])
                        else:
                            nc.scalar.copy(dsl, dp[:])
                    nc.scalar.dma_start(
                        yt_ap[q * 1024:(q + 1) * 1024, t0:t0 + T]
                        .rearrange("(k p) t -> p k t", p=128),
                        da[:, q * 8 * T:(q + 1) * 8 * T]
                        .rearrange("p (k t) -> p k t", k=8),
                    )

    nc.compile()
    return nc


def _prep_consts(task_emb, task_ids, Wp, bp, centers, A, Bm, adapter_scale):
    scale = float(np.asarray(adapter_scale))
    A_all = np.ascontiguousarray(
        A.transpose(1, 0, 2).reshape(H, NB * R).astype(np.float32))
    W2 = (Wp @ centers.T).astype(np.float32)                     # [H, 128]

    # ah: [p, q, k2, two, m] = A_all[(2*k2+two)*128+p, q*128+m], fp8 e4m3
    # (DoubleRow pairs of consecutive k-tiles interleave along the free dim)
    ah = (A_all.reshape(NKT, 128, 4, 128).transpose(1, 2, 0, 3)
          .reshape(128, 4 * NKT * 128).astype(ml_dtypes.float8_e4m3))
    ah = np.ascontiguousarray(ah)
    # ws: [p, k2, two, m] = W2[(2*k2+two)*128+p, m], fp8 (DoubleRow pairs)
    wsn = np.ascontiguousarray(
        W2.reshape(NKT, 128, 128).transpose(1, 0, 2).reshape(128, NKT * 128)
        .astype(ml_dtypes.float8_e4m3))

    # block-diag up-projection, K=128 per h-chunk
    bpk = np.zeros((128, NKT * 128), np.float32)
    for hc in range(NKT):
        for mblk in range(4):
            n = hc * 4 + mblk
            for r in range(R):
                row = (hc % 8) * 16 + mblk * 4 + r
                bpk[row, hc * 128 + mblk * 32: hc * 128 + mblk * 32 + 32] = \
                    Bm[n, r, :] * scale
    bpk = bpk.astype(ml_dtypes.bfloat16)

    e_np = (np.arange(128)[:, None] == (np.arange(512)[None, :] // 4)) \
        .astype(ml_dtypes.bfloat16)

    sconst = (bp @ centers.T - 0.5 * (centers ** 2).sum(-1)).astype(np.float32)

    biases = []
    for c in range(NCORES):
        te = task_emb[int(np.asarray(task_ids)[c // 2])].astype(np.float32)
        b5 = np.empty((128, 5), np.float32)
        zoff = te @ A_all                                        # [512]
        for q in range(4):
            b5[:, q] = zoff[q * 128:(q + 1) * 128]
        b5[:, 4] = te @ W2 + sconst
        biases.append(np.ascontiguousarray(b5))
    return ah, wsn, bpk, e_np, biases


def kernel(x, task_ids, task_emb, Wp, bp, centers, A, Bm, adapter_scale):
    global _COMPILED, LAST_RESULT
    from concourse import bass_utils

    x = np.asarray(x, dtype=np.float32)
    task_ids = np.asarray(task_ids)
    task_emb = np.asarray(task_emb, dtype=np.float32)
    Wp = np.asarray(Wp, dtype=np.float32)
    bp = np.asarray(bp, dtype=np.float32)
    centers = np.asarray(centers, dtype=np.float32)
    A = np.asarray(A, dtype=np.float32)
    Bm = np.asarray(Bm, dtype=np.float32)

    if _COMPILED is None:
        _COMPILED = _build()
    nc = _COMPILED

    ah, wsn, bpk, e_np, biases = _prep_consts(
        task_emb, task_ids, Wp, bp, centers, A, Bm, adapter_scale)

    xf = x.reshape(B * S, H)
    xf8 = xf.astype(ml_dtypes.float8_e4m3)
    in_maps = []
    for c in range(NCORES):
        xtc = np.ascontiguousarray(xf8[c * TPC:(c + 1) * TPC].T)
        in_maps.append({"xt": xtc, "ah": ah, "ws": wsn, "bpk": bpk,
                       "e": e_np, "bias": biases[c]})

    kwargs = {}
    if TRACE:
        kwargs = dict(trace=True, tmpdir=TRACE_DIR)
    res = bass_utils.run_bass_kernel_spmd(
        nc, in_maps, core_ids=list(range(NCORES)), **kwargs)
    LAST_RESULT = res

    out = np.empty((B * S, H), np.float32)
    for c in range(NCORES):
        out[c * TPC:(c + 1) * TPC] = xf[c * TPC:(c + 1) * TPC] + \
            res.results[c]["yt"].T.astype(np.float32)
    return out.reshape(B, S, H)


# revision 11
# speedup vs baseline: 1.2190x; 1.2190x over previous
"""Trainium2 Bass kernel for NeuroplasticLlama block-sparse adapter (moe_routing).

Contract: kernel(**inputs) takes FULL unsharded inputs (as produced by
setup_inputs) and returns the FULL [4, 4096, 4096] float32 output.

Strategy (data/sequence parallel over 8 cores, 2048 tokens each):
  - Each core's 2048 contiguous tokens belong to exactly one batch, so the
    task embedding contributes only per-core constant bias vectors
    (te @ A, te @ W2) -- h = x + te is never materialized.
  - The whole routed computation is made dense:
      scores s[t,n] = x @ (Wp @ centers.T)[:,n] + const_n   (per-token shift
        dropped; softmax over top-k and the top-k set are shift invariant)
      top-3 selection via threshold = 3rd max (3 rounds of max + mask-out)
      gates g[t,n] = exp(s - max) * (s >= thr3) / sum(...)
      z[t,:] (all 512 block-rank pairs) = x @ A_all  (dense)
      zg = z * expand4(g);  delta = block-diag(Bm) matmul
  - The device computes only DELTA (fp8 in, fp8 out); the residual
    y = x + delta is applied on the host during unsharding, so the x term
    keeps full f32 precision and HBM traffic per core is 2 MB x-in +
    2 MB delta-out per 512-token macrotile.
  - x is fed pre-transposed [H, tokens] fp8e4m3; scores and z both run as
    fp8 DoubleRow matmuls off the same tile (fp32 scores cost 127us/core
    of PE time in the previous version; fp8 scores ~14us with rel err
    1.4e-3 vs the 2e-2 budget -- routing flips only happen on near-ties
    where the softmax gates make the difference negligible).
  - delta matmul is bf16 (DoubleRow would not help: it is output-stream
    bound at 512 cols/chunk), PSUM -> SBUF fp8 copies alternate between
    the Scalar and Vector engines to keep both under the PE's ~26us/mt.
"""

import sys

if "/opt/trn_rl_repo" not in sys.path:
    sys.path.insert(0, "/opt/trn_rl_repo")

import numpy as np
import ml_dtypes

H = 4096
NB = 128
BLK = 32
R = 4
B = 4
S = 4096
NCORES = 8
TPC = (B * S) // NCORES  # tokens per core = 2048
T = 512                  # tokens per macrotile
NMT = TPC // T           # 4 macrotiles per core
NKT = H // 128           # 32 k-tiles over the hidden dim
NP = NKT // 2            # 16 DoubleRow k-pair tiles
BIG = 1.0e30

TRACE = False            # set by test.py for profiling runs
TRACE_DIR = None
LAST_RESULT = None       # BassKernelResults of the last run

_COMPILED = None


def _build():
    import concourse.bacc as bacc
    import concourse.tile as tile
    from concourse import mybir, masks

    f32 = mybir.dt.float32
    bf16 = mybir.dt.bfloat16
    f8 = mybir.dt.float8e4
    AF = mybir.ActivationFunctionType
    AL = mybir.AluOpType
    AX = mybir.AxisListType
    DR = mybir.MatmulPerfMode.DoubleRow

    nc = bacc.Bacc("TRN2", target_bir_lowering=False, debug=False,
                   num_devices=NCORES)

    xt_d = nc.dram_tensor("xt", [H, TPC], f8, kind="ExternalInput")
    ah_d = nc.dram_tensor("ah", [128, 4 * NKT * 128], f8, kind="ExternalInput")
    ws_d = nc.dram_tensor("ws", [128, NKT * 128], f8, kind="ExternalInput")
    bpk_d = nc.dram_tensor("bpk", [128, NKT * 128], bf16, kind="ExternalInput")
    e_d = nc.dram_tensor("e", [128, 512], bf16, kind="ExternalInput")
    bias_d = nc.dram_tensor("bias", [128, 5], f32, kind="ExternalInput")
    yt_d = nc.dram_tensor("yt", [H, TPC], f8, kind="ExternalOutput")

    xt_ap = xt_d.ap()
    yt_ap = yt_d.ap()

    with tile.TileContext(nc) as tc:
        from contextlib import ExitStack
        with ExitStack() as ctx:
            cpool = ctx.enter_context(tc.tile_pool(name="consts", bufs=1))
            xpool = ctx.enter_context(tc.tile_pool(name="xg", bufs=2))
            dpool = ctx.enter_context(tc.tile_pool(name="dall", bufs=2))
            zpool = ctx.enter_context(tc.tile_pool(name="zb", bufs=8))
            gpool = ctx.enter_context(tc.tile_pool(name="gate", bufs=3))
            spool = ctx.enter_context(tc.tile_pool(name="scal", bufs=4))
            pp = ctx.enter_context(tc.tile_pool(name="ps", bufs=2, space="PSUM"))

            # ---- persistent constants ----
            ws = cpool.tile([128, NKT * 128], f8, name="ws", tag="ws")
            nc.gpsimd.dma_start(ws[:], ws_d.ap()[:])
            az = []
            for q in range(4):
                t_az = cpool.tile([128, NKT * 128], f8, name=f"az{q}",
                                  tag=f"az{q}")
                nc.gpsimd.dma_start(t_az[:], ah_d.ap()[:, q * NKT * 128:(q + 1) * NKT * 128])
                az.append(t_az)
            bpk = cpool.tile([128, NKT * 128], bf16, name="bpk", tag="bpk")
            nc.gpsimd.dma_start(bpk[:], bpk_d.ap()[:])
            esb = cpool.tile([128, 512], bf16, name="esb", tag="esb")
            nc.gpsimd.dma_start(esb[:], e_d.ap()[:])
            bias = cpool.tile([128, 5], f32, name="bias", tag="bias")
            nc.gpsimd.dma_start(bias[:], bias_d.ap()[:])
            ident = cpool.tile([128, 128], bf16, name="ident", tag="ident")
            masks.make_identity(nc, ident[:])

            NTS = T // 128  # token sub-tiles per macrotile

            for mt in range(NMT):
                t0 = mt * T
                # ---- load x macrotile (fp8, one 2MB DMA) ----
                xa = xpool.tile([128, NKT * T], f8, name="xa", tag="xa")
                nc.sync.dma_start(
                    xa.rearrange("p (k t) -> p k t", k=NKT),
                    xt_ap[:, t0:t0 + T].rearrange("(k p) t -> p k t", p=128),
                )
                # delta output staging tile
                da = dpool.tile([128, NKT * T], f8, name="da", tag="da")

                # ---- scores chunk (fp8 DoubleRow): sT[n, t] ----
                sp = pp.tile([128, T], f32, space="PSUM", name="sp", tag="zp")
                for k2 in range(NP):
                    nc.tensor.matmul(
                        sp[:],
                        ws[:, k2 * 256:(k2 + 1) * 256]
                        .rearrange("p (two m) -> p two m", two=2),
                        xa[:, 2 * k2 * T:(2 * k2 + 2) * T]
                        .rearrange("p (two t) -> p two t", two=2),
                        start=(k2 == 0), stop=(k2 == NP - 1),
                        perf_mode=DR,
                    )
                s_sb = gpool.tile([128, T], bf16, name="s_sb", tag="s_sb")
                nc.scalar.activation(s_sb[:], sp[:], AF.Identity,
                                     bias=bias[:, 4:5], scale=1.0)

                # ---- gating phase A: transpose score subtiles ----
                # all 4 transposes land in one half-bank PSUM tile, then one
                # scalar copy moves them out (fewer ops, fewer sem hops)
                s_ps = pp.tile([128, T], bf16, space="PSUM", name="s_ps",
                               tag="tr", bufs=1)
                for ts in range(NTS):
                    nc.tensor.transpose(s_ps[:, ts * 128:(ts + 1) * 128],
                                        s_sb[:, ts * 128:(ts + 1) * 128],
                                        ident[:])
                stn_all = gpool.tile([128, T], f32, name="stn_all", tag="stn",
                                     bufs=2)
                nc.scalar.copy(stn_all[:], s_ps[:])
                stns = [stn_all[:, ts * 128:(ts + 1) * 128] for ts in range(NTS)]

                # ---- gating phase B: DVE chain (overlaps z matmuls) ----
                ggs = []
                for ts in range(NTS):
                    stn = stns[ts]
                    r1 = spool.tile([128, 1], f32, name="r1", tag="r1")
                    nc.vector.reduce_max(r1[:], stn, axis=AX.X)
                    mb1 = gpool.tile([128, 128], f32, name="mb1", tag="mb1")
                    nc.vector.tensor_scalar(mb1[:], stn, r1[:], BIG,
                                            AL.is_ge, AL.mult)
                    s2 = gpool.tile([128, 128], f32, name="s2", tag="s2")
                    nc.vector.tensor_sub(s2[:], stn, mb1[:])
                    r2 = spool.tile([128, 1], f32, name="r2", tag="r2")
                    nc.vector.reduce_max(r2[:], s2[:], axis=AX.X)
                    mb2 = gpool.tile([128, 128], f32, name="mb2", tag="mb2")
                    nc.vector.tensor_scalar(mb2[:], s2[:], r2[:], BIG,
                                            AL.is_ge, AL.mult)
                    s3 = gpool.tile([128, 128], f32, name="s3", tag="s3")
                    nc.vector.tensor_sub(s3[:], s2[:], mb2[:])
                    r3 = spool.tile([128, 1], f32, name="r3", tag="r3")
                    nc.vector.reduce_max(r3[:], s3[:], axis=AX.X)
                    nr1 = spool.tile([128, 1], f32, name="nr1", tag="nr1")
                    nc.vector.tensor_scalar_mul(nr1[:], r1[:], -1.0)
                    ex = gpool.tile([128, 128], f32, name="ex", tag="ex")
                    nc.scalar.activation(ex[:], stn, AF.Exp, bias=nr1[:],
                                         scale=1.0)
                    em = gpool.tile([128, 128], f32, name="em", tag="em")
                    zs = spool.tile([128, 1], f32, name="zs", tag="zs")
                    nc.vector.scalar_tensor_tensor(em[:], stn, r3[:], ex[:],
                                                   AL.is_ge, AL.mult,
                                                   accum_out=zs[:])
                    rz = spool.tile([128, 1], f32, name="rz", tag="rz")
                    nc.vector.reciprocal(rz[:], zs[:])
                    gg = gpool.tile([128, 128], bf16, name="gg", tag="gg",
                                    bufs=NTS + 1)
                    nc.vector.tensor_scalar_mul(gg[:], em[:], rz[:])
                    ggs.append(gg)

                # ---- z chunks (fp8 DoubleRow) ----
                zbs = []
                for q in range(4):
                    zp = pp.tile([128, T], f32, space="PSUM", name="zp", tag="zp")
                    for k2 in range(NP):
                        nc.tensor.matmul(
                            zp[:],
                            az[q][:, k2 * 256:(k2 + 1) * 256]
                            .rearrange("p (two m) -> p two m", two=2),
                            xa[:, 2 * k2 * T:(2 * k2 + 2) * T]
                            .rearrange("p (two t) -> p two t", two=2),
                            start=(k2 == 0), stop=(k2 == NP - 1),
                            perf_mode=DR,
                        )
                    zb = zpool.tile([128, T], bf16, name="zb", tag="zb")
                    nc.scalar.activation(zb[:], zp[:], AF.Identity,
                                         bias=bias[:, q:q + 1], scale=1.0)
                    zbs.append(zb)

                # ---- gating phase C: transpose gates back ----
                g_ps = pp.tile([128, T], bf16, space="PSUM", name="g_ps",
                               tag="tr", bufs=1)
                for ts in range(NTS):
                    nc.tensor.transpose(g_ps[:, ts * 128:(ts + 1) * 128],
                                        ggs[ts][:], ident[:])
                gt_sb = gpool.tile([128, T], bf16, name="gt_sb", tag="gt_sb")
                nc.scalar.copy(gt_sb[:], g_ps[:])

                # ---- per quarter: expand gates, apply, delta, store ----
                # zg = (zp + bias) * gx in one fused DVE op; the 32
                # PSUM->SBUF fp8 copies split 20 scalar / 12 vector
                VEC_HL = {2, 5, 7}
                for q in range(4):
                    gx = pp.tile([128, T], f32, space="PSUM", name="gx", tag="gx",
                                 bufs=1)
                    nc.tensor.matmul(gx[:],
                                     esb[:, q * 128:(q + 1) * 128],
                                     gt_sb[:],
                                     start=True, stop=True)
                    nc.vector.tensor_mul(zbs[q][:], zbs[q][:], gx[:])
                    for hl in range(8):
                        hc = q * 8 + hl
                        dp = pp.tile([128, T], f32, space="PSUM", name="dp",
                                     tag="dp", bufs=4)
                        nc.tensor.matmul(dp[:],
                                         bpk[:, hc * 128:(hc + 1) * 128],
                                         zbs[q][:],
                                         start=True, stop=True)
                        dsl = da[:, hc * T:(hc + 1) * T]
                        if hl in VEC_HL:
                            nc.vector.tensor_copy(dsl, dp[:])
                        else:
                            nc.scalar.copy(dsl, dp[:])
                    nc.scalar.dma_start(
                        yt_ap[q * 1024:(q + 1) * 1024, t0:t0 + T]
                        .rearrange("(k p) t -> p k t", p=128),
                        da[:, q * 8 * T:(q + 1) * 8 * T]
                        .rearrange("p (k t) -> p k t", k=8),
                    )

    nc.compile()
    return nc


def _prep_consts(task_emb, task_ids, Wp, bp, centers, A, Bm, adapter_scale):
    scale = float(np.asarray(adapter_scale))
    A_all = np.ascontiguousarray(
        A.transpose(1, 0, 2).reshape(H, NB * R).astype(np.float32))
    W2 = (Wp @ centers.T).astype(np.float32)                     # [H, 128]

    # ah: [p, q, k2, two, m] = A_all[(2*k2+two)*128+p, q*128+m], fp8 e4m3
    # (DoubleRow pairs of consecutive k-tiles interleave along the free dim)
    ah = (A_all.reshape(NKT, 128, 4, 128).transpose(1, 2, 0, 3)
          .reshape(128, 4 * NKT * 128).astype(ml_dtypes.float8_e4m3))
    ah = np.ascontiguousarray(ah)
    # ws: [p, k2, two, m] = W2[(2*k2+two)*128+p, m], fp8 (DoubleRow pairs)
    wsn = np.ascontiguousarray(
        W2.reshape(NKT, 128, 128).transpose(1, 0, 2).reshape(128, NKT * 128)
        .astype(ml_dtypes.float8_e4m3))

    # block-diag up-projection, K=128 per h-chunk
    bpk = np.zeros((128, NKT * 128), np.float32)
    for hc in range(NKT):
        for mblk in range(4):
            n = hc * 4 + mblk
            for r in range(R):
                row = (hc % 8) * 16 + mblk * 4 + r
                bpk[row, hc * 128 + mblk * 32: hc * 128 + mblk * 32 + 32] = \
                    Bm[n, r, :] * scale
    bpk = bpk.astype(ml_dtypes.bfloat16)

    e_np = (np.arange(128)[:, None] == (np.arange(512)[None, :] // 4)) \
        .astype(ml_dtypes.bfloat16)

    sconst = (bp @ centers.T - 0.5 * (centers ** 2).sum(-1)).astype(np.float32)

    biases = []
    for c in range(NCORES):
        te = task_emb[int(np.asarray(task_ids)[c // 2])].astype(np.float32)
        b5 = np.empty((128, 5), np.float32)
        zoff = te @ A_all                                        # [512]
        for q in range(4):
            b5[:, q] = zoff[q * 128:(q + 1) * 128]
        b5[:, 4] = te @ W2 + sconst
        biases.append(np.ascontiguousarray(b5))
    return ah, wsn, bpk, e_np, biases


def kernel(x, task_ids, task_emb, Wp, bp, centers, A, Bm, adapter_scale):
    global _COMPILED, LAST_RESULT
    from concourse import bass_utils

    x = np.asarray(x, dtype=np.float32)
    task_ids = np.asarray(task_ids)
    task_emb = np.asarray(task_emb, dtype=np.float32)
    Wp = np.asarray(Wp, dtype=np.float32)
    bp = np.asarray(bp, dtype=np.float32)
    centers = np.asarray(centers, dtype=np.float32)
    A = np.asarray(A, dtype=np.float32)
    Bm = np.asarray(Bm, dtype=np.float32)

    if _COMPILED is None:
        _COMPILED = _build()
    nc = _COMPILED

    ah, wsn, bpk, e_np, biases = _prep_consts(
        task_emb, task_ids, Wp, bp, centers, A, Bm, adapter_scale)

    xf = x.reshape(B * S, H)
    xf8 = xf.astype(ml_dtypes.float8_e4m3)
    in_maps = []
    for c in range(NCORES):
        xtc = np.ascontiguousarray(xf8[c * TPC:(c + 1) * TPC].T)
        in_maps.append({"xt": xtc, "ah": ah, "ws": wsn, "bpk": bpk,
                       "e": e_np, "bias": biases[c]})

    kwargs = {}
    if TRACE:
        kwargs = dict(trace=True, tmpdir=TRACE_DIR)
    res = bass_utils.run_bass_kernel_spmd(
        nc, in_maps, core_ids=list(range(NCORES)), **kwargs)
    LAST_RESULT = res

    out = np.empty((B * S, H), np.float32)
    for c in range(NCORES):
        out[c * TPC:(c + 1) * TPC] = xf[c * TPC:(c + 1) * TPC] + \
            res.results[c]["yt"].T.astype(np.float32)
    return out.reshape(B, S, H)


# revision 12
# speedup vs baseline: 1.3760x; 1.1287x over previous
"""Trainium2 Bass kernel for NeuroplasticLlama block-sparse adapter (moe_routing).

Contract: kernel(**inputs) takes FULL unsharded inputs (as produced by
setup_inputs) and returns the FULL [4, 4096, 4096] float32 output.

Strategy (data/sequence parallel over 8 cores, 2048 tokens each):
  - Each core's 2048 contiguous tokens belong to exactly one batch, so the
    task embedding contributes only per-core constant bias vectors
    (te @ A, te @ W2) -- h = x + te is never materialized.
  - The whole routed computation is made dense:
      scores s[t,n] = x @ (Wp @ centers.T)[:,n] + const_n   (per-token shift
        dropped; softmax over top-k and the top-k set are shift invariant)
      top-3 selection via threshold = 3rd max (3 rounds of max + mask-out)
      gates g[t,n] = exp(s - max) * (s >= thr3) / sum(...)
      z[t,:] (all 512 block-rank pairs) = x @ A_all  (dense)
      zg = z * expand4(g);  delta = block-diag(Bm) matmul
  - The device computes only DELTA (fp8 in, fp8 out); the residual
    y = x + delta is applied on the host during unsharding, so the x term
    keeps full f32 precision and HBM traffic per core is 2 MB x-in +
    2 MB delta-out per 512-token macrotile.
  - x is fed pre-transposed [H, tokens] fp8e4m3; scores and z both run as
    fp8 DoubleRow matmuls off the same tile (fp32 scores cost 127us/core
    of PE time in the previous version; fp8 scores ~14us with rel err
    1.4e-3 vs the 2e-2 budget -- routing flips only happen on near-ties
    where the softmax gates make the difference negligible).
  - delta matmul is bf16 (DoubleRow would not help: it is output-stream
    bound at 512 cols/chunk), PSUM -> SBUF fp8 copies alternate between
    the Scalar and Vector engines to keep both under the PE's ~26us/mt.
"""

import sys

if "/opt/trn_rl_repo" not in sys.path:
    sys.path.insert(0, "/opt/trn_rl_repo")

import numpy as np
import ml_dtypes

H = 4096
NB = 128
BLK = 32
R = 4
B = 4
S = 4096
NCORES = 8
TPC = (B * S) // NCORES  # tokens per core = 2048
T = 512                  # tokens per macrotile
NMT = TPC // T           # 4 macrotiles per core
NKT = H // 128           # 32 k-tiles over the hidden dim
NP = NKT // 2            # 16 DoubleRow k-pair tiles
BIG = 1.0e30

TRACE = False            # set by test.py for profiling runs
TRACE_DIR = None
LAST_RESULT = None       # BassKernelResults of the last run

_COMPILED = None


def _build():
    import concourse.bacc as bacc
    import concourse.tile as tile
    from concourse import mybir, masks

    f32 = mybir.dt.float32
    bf16 = mybir.dt.bfloat16
    f8 = mybir.dt.float8e4
    AF = mybir.ActivationFunctionType
    AL = mybir.AluOpType
    AX = mybir.AxisListType
    DR = mybir.MatmulPerfMode.DoubleRow

    nc = bacc.Bacc("TRN2", target_bir_lowering=False, debug=False,
                   num_devices=NCORES)

    # xt/yt are laid out host-side as [p][mt][k][t] so every DMA is a
    # contiguous [128, N] copy (128 descriptors; the naive [H, TPC] slice
    # pattern cost ~18us of HWDGE descriptor generation per load)
    xt_d = nc.dram_tensor("xt", [128, NMT * NKT * T], f8, kind="ExternalInput")
    ah_d = nc.dram_tensor("ah", [128, 4 * NKT * 128], f8, kind="ExternalInput")
    ws_d = nc.dram_tensor("ws", [128, NKT * 128], f8, kind="ExternalInput")
    bpk_d = nc.dram_tensor("bpk", [128, NKT * 128], bf16, kind="ExternalInput")
    e_d = nc.dram_tensor("e", [128, 512], bf16, kind="ExternalInput")
    bias_d = nc.dram_tensor("bias", [128, 5], f32, kind="ExternalInput")
    yt_d = nc.dram_tensor("yt", [128, NMT * NKT * T], f8, kind="ExternalOutput")

    xt_ap = xt_d.ap()
    yt_ap = yt_d.ap()

    with tile.TileContext(nc) as tc:
        from contextlib import ExitStack
        with ExitStack() as ctx:
            cpool = ctx.enter_context(tc.tile_pool(name="consts", bufs=1))
            xpool = ctx.enter_context(tc.tile_pool(name="xg", bufs=2))
            dpool = ctx.enter_context(tc.tile_pool(name="dall", bufs=2))
            zpool = ctx.enter_context(tc.tile_pool(name="zb", bufs=8))
            gpool = ctx.enter_context(tc.tile_pool(name="gate", bufs=3))
            spool = ctx.enter_context(tc.tile_pool(name="scal", bufs=4))
            pp = ctx.enter_context(tc.tile_pool(name="ps", bufs=2, space="PSUM"))

            # ---- persistent constants ----
            ws = cpool.tile([128, NKT * 128], f8, name="ws", tag="ws")
            nc.gpsimd.dma_start(ws[:], ws_d.ap()[:])
            az = []
            for q in range(4):
                t_az = cpool.tile([128, NKT * 128], f8, name=f"az{q}",
                                  tag=f"az{q}")
                nc.gpsimd.dma_start(t_az[:], ah_d.ap()[:, q * NKT * 128:(q + 1) * NKT * 128])
                az.append(t_az)
            bpk = cpool.tile([128, NKT * 128], bf16, name="bpk", tag="bpk")
            nc.gpsimd.dma_start(bpk[:], bpk_d.ap()[:])
            esb = cpool.tile([128, 512], bf16, name="esb", tag="esb")
            nc.gpsimd.dma_start(esb[:], e_d.ap()[:])
            bias = cpool.tile([128, 5], f32, name="bias", tag="bias")
            nc.gpsimd.dma_start(bias[:], bias_d.ap()[:])
            ident = cpool.tile([128, 128], bf16, name="ident", tag="ident")
            masks.make_identity(nc, ident[:])

            NTS = T // 128  # token sub-tiles per macrotile

            for mt in range(NMT):
                t0 = mt * T
                # ---- load x macrotile (fp8, one contiguous 2MB DMA) ----
                xa = xpool.tile([128, NKT * T], f8, name="xa", tag="xa")
                nc.sync.dma_start(
                    xa[:], xt_ap[:, mt * NKT * T:(mt + 1) * NKT * T])
                # delta output staging tile
                da = dpool.tile([128, NKT * T], f8, name="da", tag="da")

                # ---- scores chunk (fp8 DoubleRow): sT[n, t] ----
                sp = pp.tile([128, T], f32, space="PSUM", name="sp", tag="zp")
                for k2 in range(NP):
                    nc.tensor.matmul(
                        sp[:],
                        ws[:, k2 * 256:(k2 + 1) * 256]
                        .rearrange("p (two m) -> p two m", two=2),
                        xa[:, 2 * k2 * T:(2 * k2 + 2) * T]
                        .rearrange("p (two t) -> p two t", two=2),
                        start=(k2 == 0), stop=(k2 == NP - 1),
                        perf_mode=DR,
                    )
                s_sb = gpool.tile([128, T], bf16, name="s_sb", tag="s_sb")
                nc.scalar.activation(s_sb[:], sp[:], AF.Identity,
                                     bias=bias[:, 4:5], scale=1.0)

                # ---- gating phase A: transpose score subtiles ----
                # all 4 transposes land in one half-bank PSUM tile, then one
                # scalar copy moves them out (fewer ops, fewer sem hops)
                s_ps = pp.tile([128, T], bf16, space="PSUM", name="s_ps",
                               tag="tr", bufs=1)
                for ts in range(NTS):
                    nc.tensor.transpose(s_ps[:, ts * 128:(ts + 1) * 128],
                                        s_sb[:, ts * 128:(ts + 1) * 128],
                                        ident[:])
                stn_all = gpool.tile([128, T], f32, name="stn_all", tag="stn",
                                     bufs=2)
                nc.scalar.copy(stn_all[:], s_ps[:])
                stns = [stn_all[:, ts * 128:(ts + 1) * 128] for ts in range(NTS)]

                # ---- gating phase B: DVE chain (overlaps z matmuls) ----
                ggs = []
                for ts in range(NTS):
                    stn = stns[ts]
                    r1 = spool.tile([128, 1], f32, name="r1", tag="r1")
                    nc.vector.reduce_max(r1[:], stn, axis=AX.X)
                    mb1 = gpool.tile([128, 128], f32, name="mb1", tag="mb1")
                    nc.vector.tensor_scalar(mb1[:], stn, r1[:], BIG,
                                            AL.is_ge, AL.mult)
                    s2 = gpool.tile([128, 128], f32, name="s2", tag="s2")
                    nc.vector.tensor_sub(s2[:], stn, mb1[:])
                    r2 = spool.tile([128, 1], f32, name="r2", tag="r2")
                    nc.vector.reduce_max(r2[:], s2[:], axis=AX.X)
                    mb2 = gpool.tile([128, 128], f32, name="mb2", tag="mb2")
                    nc.vector.tensor_scalar(mb2[:], s2[:], r2[:], BIG,
                                            AL.is_ge, AL.mult)
                    s3 = gpool.tile([128, 128], f32, name="s3", tag="s3")
                    nc.vector.tensor_sub(s3[:], s2[:], mb2[:])
                    r3 = spool.tile([128, 1], f32, name="r3", tag="r3")
                    nc.vector.reduce_max(r3[:], s3[:], axis=AX.X)
                    nr1 = spool.tile([128, 1], f32, name="nr1", tag="nr1")
                    nc.vector.tensor_scalar_mul(nr1[:], r1[:], -1.0)
                    ex = gpool.tile([128, 128], f32, name="ex", tag="ex")
                    nc.scalar.activation(ex[:], stn, AF.Exp, bias=nr1[:],
                                         scale=1.0)
                    em = gpool.tile([128, 128], f32, name="em", tag="em")
                    zs = spool.tile([128, 1], f32, name="zs", tag="zs")
                    nc.vector.scalar_tensor_tensor(em[:], stn, r3[:], ex[:],
                                                   AL.is_ge, AL.mult,
                                                   accum_out=zs[:])
                    rz = spool.tile([128, 1], f32, name="rz", tag="rz")
                    nc.vector.reciprocal(rz[:], zs[:])
                    gg = gpool.tile([128, 128], bf16, name="gg", tag="gg",
                                    bufs=NTS + 1)
                    nc.vector.tensor_scalar_mul(gg[:], em[:], rz[:])
                    ggs.append(gg)

                # ---- z chunks (fp8 DoubleRow) ----
                zbs = []
                for q in range(4):
                    zp = pp.tile([128, T], f32, space="PSUM", name="zp", tag="zp")
                    for k2 in range(NP):
                        nc.tensor.matmul(
                            zp[:],
                            az[q][:, k2 * 256:(k2 + 1) * 256]
                            .rearrange("p (two m) -> p two m", two=2),
                            xa[:, 2 * k2 * T:(2 * k2 + 2) * T]
                            .rearrange("p (two t) -> p two t", two=2),
                            start=(k2 == 0), stop=(k2 == NP - 1),
                            perf_mode=DR,
                        )
                    zb = zpool.tile([128, T], bf16, name="zb", tag="zb")
                    nc.scalar.activation(zb[:], zp[:], AF.Identity,
                                         bias=bias[:, q:q + 1], scale=1.0)
                    zbs.append(zb)

                # ---- gating phase C: transpose gates back ----
                g_ps = pp.tile([128, T], bf16, space="PSUM", name="g_ps",
                               tag="tr", bufs=1)
                for ts in range(NTS):
                    nc.tensor.transpose(g_ps[:, ts * 128:(ts + 1) * 128],
                                        ggs[ts][:], ident[:])
                gt_sb = gpool.tile([128, T], bf16, name="gt_sb", tag="gt_sb")
                nc.scalar.copy(gt_sb[:], g_ps[:])

                # ---- per quarter: expand gates, apply, delta, store ----
                # zg = (zp + bias) * gx in one fused DVE op; the 32
                # PSUM->SBUF fp8 copies split 20 scalar / 12 vector
                VEC_HL = {2, 5, 7}
                for q in range(4):
                    gx = pp.tile([128, T], f32, space="PSUM", name="gx", tag="gx",
                                 bufs=1)
                    nc.tensor.matmul(gx[:],
                                     esb[:, q * 128:(q + 1) * 128],
                                     gt_sb[:],
                                     start=True, stop=True)
                    nc.vector.tensor_mul(zbs[q][:], zbs[q][:], gx[:])
                    for hl in range(8):
                        hc = q * 8 + hl
                        dp = pp.tile([128, T], f32, space="PSUM", name="dp",
                                     tag="dp", bufs=4)
                        nc.tensor.matmul(dp[:],
                                         bpk[:, hc * 128:(hc + 1) * 128],
                                         zbs[q][:],
                                         start=True, stop=True)
                        dsl = da[:, hc * T:(hc + 1) * T]
                        if hl in VEC_HL:
                            nc.vector.tensor_copy(dsl, dp[:])
                        else:
                            nc.scalar.copy(dsl, dp[:])
                    nc.sync.dma_start(
                        yt_ap[:, (mt * 4 + q) * 8 * T:(mt * 4 + q + 1) * 8 * T],
                        da[:, q * 8 * T:(q + 1) * 8 * T],
                    )

    nc.compile()
    return nc


def _prep_consts(task_emb, task_ids, Wp, bp, centers, A, Bm, adapter_scale):
    scale = float(np.asarray(adapter_scale))
    A_all = np.ascontiguousarray(
        A.transpose(1, 0, 2).reshape(H, NB * R).astype(np.float32))
    W2 = (Wp @ centers.T).astype(np.float32)                     # [H, 128]

    # ah: [p, q, k2, two, m] = A_all[(2*k2+two)*128+p, q*128+m], fp8 e4m3
    # (DoubleRow pairs of consecutive k-tiles interleave along the free dim)
    ah = (A_all.reshape(NKT, 128, 4, 128).transpose(1, 2, 0, 3)
          .reshape(128, 4 * NKT * 128).astype(ml_dtypes.float8_e4m3))
    ah = np.ascontiguousarray(ah)
    # ws: [p, k2, two, m] = W2[(2*k2+two)*128+p, m], fp8 (DoubleRow pairs)
    wsn = np.ascontiguousarray(
        W2.reshape(NKT, 128, 128).transpose(1, 0, 2).reshape(128, NKT * 128)
        .astype(ml_dtypes.float8_e4m3))

    # block-diag up-projection, K=128 per h-chunk
    bpk = np.zeros((128, NKT * 128), np.float32)
    for hc in range(NKT):
        for mblk in range(4):
            n = hc * 4 + mblk
            for r in range(R):
                row = (hc % 8) * 16 + mblk * 4 + r
                bpk[row, hc * 128 + mblk * 32: hc * 128 + mblk * 32 + 32] = \
                    Bm[n, r, :] * scale
    bpk = bpk.astype(ml_dtypes.bfloat16)

    e_np = (np.arange(128)[:, None] == (np.arange(512)[None, :] // 4)) \
        .astype(ml_dtypes.bfloat16)

    sconst = (bp @ centers.T - 0.5 * (centers ** 2).sum(-1)).astype(np.float32)

    biases = []
    for c in range(NCORES):
        te = task_emb[int(np.asarray(task_ids)[c // 2])].astype(np.float32)
        b5 = np.empty((128, 5), np.float32)
        zoff = te @ A_all                                        # [512]
        for q in range(4):
            b5[:, q] = zoff[q * 128:(q + 1) * 128]
        b5[:, 4] = te @ W2 + sconst
        biases.append(np.ascontiguousarray(b5))
    return ah, wsn, bpk, e_np, biases


def kernel(x, task_ids, task_emb, Wp, bp, centers, A, Bm, adapter_scale):
    global _COMPILED, LAST_RESULT
    from concourse import bass_utils

    x = np.asarray(x, dtype=np.float32)
    task_ids = np.asarray(task_ids)
    task_emb = np.asarray(task_emb, dtype=np.float32)
    Wp = np.asarray(Wp, dtype=np.float32)
    bp = np.asarray(bp, dtype=np.float32)
    centers = np.asarray(centers, dtype=np.float32)
    A = np.asarray(A, dtype=np.float32)
    Bm = np.asarray(Bm, dtype=np.float32)

    if _COMPILED is None:
        _COMPILED = _build()
    nc = _COMPILED

    ah, wsn, bpk, e_np, biases = _prep_consts(
        task_emb, task_ids, Wp, bp, centers, A, Bm, adapter_scale)

    xf = x.reshape(B * S, H)
    xf8 = xf.astype(ml_dtypes.float8_e4m3)
    in_maps = []
    for c in range(NCORES):
        xtc = xf8[c * TPC:(c + 1) * TPC].reshape(NMT, T, NKT, 128)
        xtc = np.ascontiguousarray(xtc.transpose(3, 0, 2, 1)) \
            .reshape(128, NMT * NKT * T)
        in_maps.append({"xt": xtc, "ah": ah, "ws": wsn, "bpk": bpk,
                       "e": e_np, "bias": biases[c]})

    kwargs = {}
    if TRACE:
        kwargs = dict(trace=True, tmpdir=TRACE_DIR)
    res = bass_utils.run_bass_kernel_spmd(
        nc, in_maps, core_ids=list(range(NCORES)), **kwargs)
    LAST_RESULT = res

    out = np.empty((B * S, H), np.float32)
    for c in range(NCORES):
        dat = res.results[c]["yt"].reshape(128, NMT, 4, 8, T)
        delta = dat.transpose(1, 4, 2, 3, 0).reshape(TPC, H)
        out[c * TPC:(c + 1) * TPC] = xf[c * TPC:(c + 1) * TPC] + \
            delta.astype(np.float32)
    return out.reshape(B, S, H)
